# revision 3
# baseline (speedup 1.0000x reference)
"""Trainium2 Bass kernel for the unrolled-GRU + FC-head problem.

Math (per example b):
    gi[t] = x[t] @ w_ih.T + b_ih                       # [T, 3H]
    gh    = h  @ w_hh.T + b_hh                         # per step
    r = sig(gi_r + gh_r); z = sig(gi_z + gh_z)
    n = tanh(gi_n + r * gh_n)
    h = (1 - z) * n + z * h                            # T sequential steps
    out = relu(h @ w_fc1.T + b_fc1) @ w_fc2.T + b_fc2  # [C]

Sharding: data-parallel over batch. B=512 over 8 cores -> B_local=64.

Per-core design (matmul operands are float32r -- full-rate 1 col/cycle on
the PE for N>=256, measured ~7e-5 relative error, far better than tf32):
  - batch is the matmul *stationary* operand: lhsT = h^T chunk [K=128, M=64],
    weights stream as the moving operand (rhs = w^T [128, N<=512]).
  - PSUM G    [64,1536]: b_ih+b_hh (r,z) + x-proj + h-proj accumulated
  - PSUM Gin  [64, 768]: b_ih(n) + x-proj(n)
  - PSUM Ghn  [64, 768]: b_hh(n) + h-proj(n)
  - biases folded into PSUM via rank-1 matmuls (ones[1,64] stationary).
  - h^T for the next step produced by 6 PE transposes + one DVE copy
    (the copy also performs the f32 -> f32r rounding the verifier needs).
"""

import os
import sys

import numpy as np

if "/opt/trn_rl_repo" not in sys.path:
    sys.path.insert(0, "/opt/trn_rl_repo")

B, T, I, H, F1, C = 512, 128, 128, 768, 256, 10
NCORES = 8
BL = B // NCORES  # 64
G3 = 3 * H  # 2304
H2 = 2 * H  # 1536
KC = H // 128  # 6 k-chunks of the hidden dim

_CACHE = {}


def _build_program():
    import concourse.bacc as bacc
    import concourse.mybir as mybir
    import concourse.tile as tile
    from concourse.masks import make_identity

    f32 = mybir.dt.float32
    f32r = mybir.dt.float32r
    AF = mybir.ActivationFunctionType

    nc = bacc.Bacc(
        "TRN2",
        target_bir_lowering=False,
        debug=False,
        enable_asserts=False,
        num_devices=NCORES,
    )

    def mm(out, lhsT, rhs, start, stop):
        nc.tensor.matmul(out, lhsT, rhs, start=start, stop=stop)

    # ---- DRAM I/O (f32r tensors carry plain fp32 bytes from numpy) ----
    xT_d = nc.dram_tensor("xT", [128, T * BL], f32r, kind="ExternalInput")
    whhT_d = nc.dram_tensor("whhT", [128, KC * G3], f32r, kind="ExternalInput")
    wihT_d = nc.dram_tensor("wihT", [128, G3], f32r, kind="ExternalInput")
    brz_d = nc.dram_tensor("brz", [1, H2], f32r, kind="ExternalInput")
    bin_d = nc.dram_tensor("bin", [1, H], f32r, kind="ExternalInput")
    bhn_d = nc.dram_tensor("bhn", [1, H], f32r, kind="ExternalInput")
    ones_d = nc.dram_tensor("ones", [1, BL], f32r, kind="ExternalInput")
    wfc1T_d = nc.dram_tensor("wfc1T", [128, KC * F1], f32r, kind="ExternalInput")
    bfc1_d = nc.dram_tensor("bfc1", [1, F1], f32r, kind="ExternalInput")
    wfc2T_d = nc.dram_tensor("wfc2T", [128, 2 * C], f32r, kind="ExternalInput")
    bfc2_d = nc.dram_tensor("bfc2", [1, C], f32r, kind="ExternalInput")
    out_d = nc.dram_tensor("logits", [BL, C], f32, kind="ExternalOutput")

    with tile.TileContext(nc) as tc:
        with (
            tc.tile_pool(name="const", bufs=1) as const,
            tc.tile_pool(name="state", bufs=2) as state,
            tc.tile_pool(name="work", bufs=2) as work,
            tc.tile_pool(name="gpsum", bufs=1, space="PSUM") as gpsum,
            tc.tile_pool(name="tpsum", bufs=1, space="PSUM") as tpsum,
        ):
            # ---- constants: DMA everything in once ----
            def load(name, shape, dram):
                t_ = const.tile(shape, f32r, tag=name)
                nc.sync.dma_start(out=t_[:], in_=dram.ap())
                return t_

            xT = load("xT", [128, T * BL], xT_d)
            wihT = load("wihT", [128, G3], wihT_d)
            brz = load("brz", [1, H2], brz_d)
            bin_ = load("bin", [1, H], bin_d)
            bhn = load("bhn", [1, H], bhn_d)
            ones = load("ones", [1, BL], ones_d)
            whhT = load("whhT", [128, KC * G3], whhT_d)
            wfc1T = load("wfc1T", [128, KC * F1], wfc1T_d)
            bfc1 = load("bfc1", [1, F1], bfc1_d)
            wfc2T = load("wfc2T", [128, 2 * C], wfc2T_d)
            bfc2 = load("bfc2", [1, C], bfc2_d)

            ident = const.tile([BL, BL], f32, tag="ident")
            make_identity(nc, ident[:])

            h_prev = None  # SBUF [64, 768] fp32
            hT = None  # SBUF [128, KC*64] f32r (transposed h)

            def transpose_h(h_sb):
                Tps = tpsum.tile([128, KC * BL], f32, tag="T")
                for k in range(KC):
                    nc.tensor.transpose(
                        Tps[:, k * BL : (k + 1) * BL],
                        h_sb[:, k * 128 : (k + 1) * 128],
                        ident[:],
                    )
                hT_new = state.tile([128, KC * BL], f32r, tag="hT")
                nc.vector.tensor_copy(hT_new[:], Tps[:])  # f32 -> f32r round
                return hT_new

            for t in range(T):
                G = gpsum.tile([BL, H2], f32, tag="G")
                Gin = gpsum.tile([BL, H], f32, tag="Gin")
                Ghn = gpsum.tile([BL, H], f32, tag="Ghn")
                xt = xT[:, t * BL : (t + 1) * BL]

                # -- PE: bias init (start=True claims each bank) --
                for c0 in range(0, H2, 512):
                    mm(G[:, c0 : c0 + 512], ones[:], brz[:, c0 : c0 + 512],
                       start=True, stop=False)
                for c0, c1 in ((0, 512), (512, 768)):
                    mm(Gin[:, c0:c1], ones[:], bin_[:, c0:c1],
                       start=True, stop=False)
                    mm(Ghn[:, c0:c1], ones[:], bhn[:, c0:c1],
                       start=True, stop=(t == 0))

                # -- PE: x projection --
                for c0 in range(0, H2, 512):
                    mm(G[:, c0 : c0 + 512], xt, wihT[:, c0 : c0 + 512],
                       start=False, stop=(t == 0))
                for c0, c1 in ((0, 512), (512, 768)):
                    mm(Gin[:, c0:c1], xt, wihT[:, H2 + c0 : H2 + c1],
                       start=False, stop=True)

                if t > 0:
                    # -- PE: transpose h_{t-1} -> hT, then h projection --
                    hT = transpose_h(h_prev)
                    for k in range(KC):
                        hk = hT[:, k * BL : (k + 1) * BL]
                        wk = k * G3
                        last = k == KC - 1
                        for c0 in range(0, H2, 512):
                            mm(G[:, c0 : c0 + 512], hk,
                               whhT[:, wk + c0 : wk + c0 + 512],
                               start=False, stop=last)
                        for c0, c1 in ((0, 512), (512, 768)):
                            mm(Ghn[:, c0:c1], hk,
                               whhT[:, wk + H2 + c0 : wk + H2 + c1],
                               start=False, stop=last)

                # -- ACT/DVE: gates + state update --
                rz = work.tile([BL, H2], f32, tag="rz")
                nc.scalar.activation(rz[:], G[:], AF.Sigmoid)
                tn = work.tile([BL, H], f32, tag="tn")
                nc.vector.tensor_mul(tn[:], rz[:, 0:H], Ghn[:])  # r*(hn+bhn)
                tn2 = work.tile([BL, H], f32, tag="tn2")
                nc.vector.tensor_add(tn2[:], tn[:], Gin[:])  # + in + bin
                n_t = work.tile([BL, H], f32, tag="n")
                nc.scalar.activation(n_t[:], tn2[:], AF.Tanh)
                h_new = state.tile([BL, H], f32, tag="h")
                if t == 0:
                    v = work.tile([BL, H], f32, tag="d")
                    nc.scalar.activation(v[:], rz[:, H:H2], AF.Copy,
                                         bias=1.0, scale=-1.0)  # 1 - z
                    nc.vector.tensor_mul(h_new[:], v[:], n_t[:])
                else:
                    d = work.tile([BL, H], f32, tag="d")
                    nc.vector.tensor_sub(d[:], h_prev[:], n_t[:])  # h - n
                    m = work.tile([BL, H], f32, tag="m")
                    nc.vector.tensor_mul(m[:], rz[:, H:H2], d[:])  # z*(h-n)
                    nc.vector.tensor_add(h_new[:], n_t[:], m[:])  # n + z*(h-n)
                h_prev = h_new

            # ---- FC head ----
            hT = transpose_h(h_prev)
            fc1 = gpsum.tile([BL, F1], f32, tag="G")
            mm(fc1[:], ones[:], bfc1[:], start=True, stop=False)
            for k in range(KC):
                mm(fc1[:], hT[:, k * BL : (k + 1) * BL],
                   wfc1T[:, k * F1 : (k + 1) * F1],
                   start=False, stop=(k == KC - 1))
            o1 = work.tile([BL, F1], f32, tag="o1")
            nc.scalar.activation(o1[:], fc1[:], AF.Relu)

            T2 = tpsum.tile([128, 2 * BL], f32, tag="T")
            nc.tensor.transpose(T2[:, 0:BL], o1[:, 0:128], ident[:])
            nc.tensor.transpose(T2[:, BL : 2 * BL], o1[:, 128:256], ident[:])
            o1T = work.tile([128, 2 * BL], f32r, tag="o1T")
            nc.vector.tensor_copy(o1T[:], T2[:])

            fc2 = gpsum.tile([BL, C], f32, tag="Gin")
            mm(fc2[:], ones[:], bfc2[:], start=True, stop=False)
            mm(fc2[:], o1T[:, 0:BL], wfc2T[:, 0:C], start=False, stop=False)
            mm(fc2[:], o1T[:, BL : 2 * BL], wfc2T[:, C : 2 * C],
               start=False, stop=True)
            lo = work.tile([BL, C], f32, tag="lo")
            nc.vector.tensor_copy(lo[:], fc2[:])
            nc.sync.dma_start(out=out_d.ap(), in_=lo[:])

    nc.compile()
    return nc


def _prep_shared(w_ih, w_hh, b_ih, b_hh, w_fc1, b_fc1, w_fc2, b_fc2):
    f = np.float32

    def kmajor(wT, kc, n):  # [kc*128, n] -> [128, kc*n]
        return np.ascontiguousarray(
            wT.reshape(kc, 128, n).transpose(1, 0, 2).reshape(128, kc * n)
        ).astype(f, copy=False)

    whhT = kmajor(np.ascontiguousarray(w_hh.T), KC, G3)
    wihT = np.ascontiguousarray(w_ih.T).astype(f, copy=False)
    b_sum = (b_ih + b_hh).astype(f)
    shared = {
        "whhT": whhT,
        "wihT": wihT,
        "brz": np.ascontiguousarray(b_sum[None, :H2]),
        "bin": np.ascontiguousarray(b_ih.astype(f)[None, H2:G3]),
        "bhn": np.ascontiguousarray(b_hh.astype(f)[None, H2:G3]),
        "ones": np.ones((1, BL), f),
        "wfc1T": kmajor(np.ascontiguousarray(w_fc1.T), KC, F1),
        "bfc1": np.ascontiguousarray(b_fc1.astype(f)[None, :]),
        "wfc2T": kmajor(np.ascontiguousarray(w_fc2.T), 2, C),
        "bfc2": np.ascontiguousarray(b_fc2.astype(f)[None, :]),
    }
    return shared


def _prep_in_maps(inputs):
    x = np.asarray(inputs["x"], dtype=np.float32)
    shared = _prep_shared(
        *(np.asarray(inputs[k], dtype=np.float32)
          for k in ("w_ih", "w_hh", "b_ih", "b_hh", "w_fc1", "b_fc1",
                    "w_fc2", "b_fc2"))
    )
    in_maps = []
    for c in range(NCORES):
        xs = x[c * BL : (c + 1) * BL]  # [64, T, I]
        xT = np.ascontiguousarray(xs.transpose(2, 1, 0).reshape(128, T * BL))
        in_maps.append({**shared, "xT": xT})
    return in_maps


def _execute(in_maps):
    from concourse.bass_utils import run_bass_kernel_spmd

    if "nc" not in _CACHE:
        _CACHE["nc"] = _build_program()
    nc = _CACHE["nc"]
    res = run_bass_kernel_spmd(nc, in_maps, core_ids=list(range(NCORES)))
    out = np.concatenate([res.results[c]["logits"] for c in range(NCORES)], axis=0)
    return out.astype(np.float32), res


def _run(inputs, trace=False, trace_kwargs=None):
    return _execute(_prep_in_maps(inputs))


def kernel(**inputs):
    out, _ = _execute(_prep_in_maps(inputs))
    return out


# revision 13
# speedup vs baseline: 834.5818x; 834.5818x over previous
"""Trainium2 Bass kernel for the unrolled-GRU + FC-head problem.

Math (per example b):
    gi[t] = x[t] @ w_ih.T + b_ih                       # [T, 3H]
    gh    = h  @ w_hh.T + b_hh                         # per step
    r = sig(gi_r + gh_r); z = sig(gi_z + gh_z)
    n = tanh(gi_n + r * gh_n)
    h = (1 - z) * n + z * h                            # T sequential steps
    out = relu(h @ w_fc1.T + b_fc1) @ w_fc2.T + b_fc2  # [C]

Sharding: data-parallel over batch. B=512 over 8 cores -> B_local=64.

Per-core design (matmul operands are float32r -- full-rate 1 col/cycle on
the PE for N>=256, measured ~7e-5 relative error, far better than tf32):
  - batch is the matmul *stationary* operand: lhsT = h^T chunk [K=128, M=64],
    weights stream as the moving operand (rhs = w^T [128, N<=512]).
  - PSUM G    [64,1536]: b_ih+b_hh (r,z) + x-proj + h-proj accumulated
  - PSUM Gin  [64, 768]: b_ih(n) + x-proj(n)
  - PSUM Ghn  [64, 768]: b_hh(n) + h-proj(n)
  - biases folded into PSUM via rank-1 matmuls (ones[1,64] stationary).
  - h^T for the next step produced by 6 PE transposes + one DVE copy
    (the copy also performs the f32 -> f32r rounding the verifier needs).
"""

import os
import sys

import numpy as np

if "/opt/trn_rl_repo" not in sys.path:
    sys.path.insert(0, "/opt/trn_rl_repo")

B, T, I, H, F1, C = 512, 128, 128, 768, 256, 10
NCORES = 8
BL = B // NCORES  # 64
G3 = 3 * H  # 2304
H2 = 2 * H  # 1536
KC = H // 128  # 6 k-chunks of the hidden dim

# v2 experiment knobs
COLTILE = os.environ.get("GRU_COLTILE", "1") == "1"
TAILOPT = os.environ.get("GRU_TAILOPT", "1") == "1"

_CACHE = {}


def _build_program(reps=1):
    import contextlib

    import concourse.bacc as bacc
    import concourse.mybir as mybir
    import concourse.tile as tile
    from concourse.masks import make_identity

    f32 = mybir.dt.float32
    f32r = mybir.dt.float32r
    AF = mybir.ActivationFunctionType

    nc = bacc.Bacc(
        "TRN2",
        target_bir_lowering=False,
        debug=False,
        enable_asserts=False,
        num_devices=NCORES,
    )

    def mm(out, lhsT, rhs, start, stop):
        """Matmul with batch (M=64) as stationary. With COLTILE, split the
        batch into two 32-column groups of the PE array: the two matmuls
        stream concurrently on separate XBUSes (disjoint output partitions),
        halving the weight-streaming wall time."""
        if not COLTILE:
            nc.tensor.matmul(out, lhsT, rhs, start=start, stop=stop)
            return
        hb = BL // 2
        nc.tensor.matmul(out[0:hb, :], lhsT[:, 0:hb], rhs,
                         start=start, stop=stop)
        nc.tensor.matmul(out[hb:BL, :], lhsT[:, hb:BL], rhs,
                         start=start, stop=stop)

    # ---- DRAM I/O (f32r tensors carry plain fp32 bytes from numpy) ----
    xT_d = nc.dram_tensor("xT", [128, T * BL], f32r, kind="ExternalInput")
    whhT_d = nc.dram_tensor("whhT", [128, KC * G3], f32r, kind="ExternalInput")
    wihT_d = nc.dram_tensor("wihT", [128, G3], f32r, kind="ExternalInput")
    brz_d = nc.dram_tensor("brz", [1, H2], f32r, kind="ExternalInput")
    bin_d = nc.dram_tensor("bin", [1, H], f32r, kind="ExternalInput")
    bhn_d = nc.dram_tensor("bhn", [1, H], f32r, kind="ExternalInput")
    ones_d = nc.dram_tensor("ones", [1, BL], f32r, kind="ExternalInput")
    wfc1T_d = nc.dram_tensor("wfc1T", [128, KC * F1], f32r, kind="ExternalInput")
    bfc1_d = nc.dram_tensor("bfc1", [1, F1], f32r, kind="ExternalInput")
    wfc2T_d = nc.dram_tensor("wfc2T", [128, 2 * C], f32r, kind="ExternalInput")
    bfc2_d = nc.dram_tensor("bfc2", [1, C], f32r, kind="ExternalInput")
    out_d = nc.dram_tensor("logits", [BL, C], f32, kind="ExternalOutput")

    with tile.TileContext(nc) as tc:
        with (
            tc.tile_pool(name="const", bufs=1) as const,
            tc.tile_pool(name="state", bufs=2) as state,
            tc.tile_pool(name="work", bufs=2) as work,
            tc.tile_pool(name="gpsum", bufs=1, space="PSUM") as gpsum,
            tc.tile_pool(name="tpsum", bufs=1, space="PSUM") as tpsum,
        ):
            # ---- constants: DMA everything in once ----
            def load(name, shape, dram):
                t_ = const.tile(shape, f32r, tag=name)
                nc.sync.dma_start(out=t_[:], in_=dram.ap())
                return t_

            xT = load("xT", [128, T * BL], xT_d)
            wihT = load("wihT", [128, G3], wihT_d)
            brz = load("brz", [1, H2], brz_d)
            bin_ = load("bin", [1, H], bin_d)
            bhn = load("bhn", [1, H], bhn_d)
            ones = load("ones", [1, BL], ones_d)
            whhT = load("whhT", [128, KC * G3], whhT_d)
            wfc1T = load("wfc1T", [128, KC * F1], wfc1T_d)
            bfc1 = load("bfc1", [1, F1], bfc1_d)
            wfc2T = load("wfc2T", [128, 2 * C], wfc2T_d)
            bfc2 = load("bfc2", [1, C], bfc2_d)

            ident = const.tile([BL, BL], f32, tag="ident")
            make_identity(nc, ident[:])

            h_prev = None  # SBUF [64, 768] fp32
            hT = None  # SBUF [128, KC*64] f32r (transposed h)

            def transpose_h(h_sb):
                Tps = tpsum.tile([128, KC * BL], f32, tag="T")
                for k in range(KC):
                    nc.tensor.transpose(
                        Tps[:, k * BL : (k + 1) * BL],
                        h_sb[:, k * 128 : (k + 1) * 128],
                        ident[:],
                    )
                hT_new = state.tile([128, KC * BL], f32r, tag="hT")
                nc.vector.tensor_copy(hT_new[:], Tps[:])  # f32 -> f32r round
                return hT_new

            def emit_body():
                emit_recurrence()
                emit_fc_head()

            def emit_recurrence():
                nonlocal h_prev, hT
                for t in range(T):
                    emit_step(t)

            def emit_step(t):
                nonlocal h_prev, hT
                G = gpsum.tile([BL, H2], f32, tag="G")
                Gin = gpsum.tile([BL, H], f32, tag="Gin")
                Ghn = gpsum.tile([BL, H], f32, tag="Ghn")
                xt = xT[:, t * BL : (t + 1) * BL]

                # -- PE: bias init (start=True claims each bank) --
                for c0 in range(0, H2, 512):
                    mm(G[:, c0 : c0 + 512], ones[:], brz[:, c0 : c0 + 512],
                       start=True, stop=False)
                for c0, c1 in ((0, 512), (512, 768)):
                    mm(Gin[:, c0:c1], ones[:], bin_[:, c0:c1],
                       start=True, stop=False)
                    mm(Ghn[:, c0:c1], ones[:], bhn[:, c0:c1],
                       start=True, stop=(t == 0))

                # -- PE: x projection --
                for c0 in range(0, H2, 512):
                    mm(G[:, c0 : c0 + 512], xt, wihT[:, c0 : c0 + 512],
                       start=False, stop=(t == 0))
                for c0, c1 in ((0, 512), (512, 768)):
                    mm(Gin[:, c0:c1], xt, wihT[:, H2 + c0 : H2 + c1],
                       start=False, stop=True)

                if t > 0:
                    # -- PE: transpose h_{t-1} -> hT, then h projection --
                    hT = transpose_h(h_prev)
                    for k in range(KC):
                        hk = hT[:, k * BL : (k + 1) * BL]
                        wk = k * G3
                        last = k == KC - 1
                        for c0 in range(0, H2, 512):
                            mm(G[:, c0 : c0 + 512], hk,
                               whhT[:, wk + c0 : wk + c0 + 512],
                               start=False, stop=last)
                        for c0, c1 in ((0, 512), (512, 768)):
                            mm(Ghn[:, c0:c1], hk,
                               whhT[:, wk + H2 + c0 : wk + H2 + c1],
                               start=False, stop=last)

                # -- ACT/DVE: gates + state update --
                h_new = state.tile([BL, H], f32, tag="h")
                if TAILOPT:
                    # critical path: r -> tn -> tn2 -> tanh -> w1 -> h_new;
                    # z / v=1-z / u=z*h run off-path in the shadow of it.
                    r_s = work.tile([BL, H], f32, tag="r")
                    nc.scalar.activation(r_s[:], G[:, 0:H], AF.Sigmoid)
                    z_s = work.tile([BL, H], f32, tag="z")
                    nc.scalar.activation(z_s[:], G[:, H:H2], AF.Sigmoid)
                    tn = work.tile([BL, H], f32, tag="tn")
                    nc.vector.tensor_mul(tn[:], r_s[:], Ghn[:])  # r*(hn+bhn)
                    tn2 = work.tile([BL, H], f32, tag="tn2")
                    nc.vector.tensor_add(tn2[:], tn[:], Gin[:])  # + in + bin
                    v = work.tile([BL, H], f32, tag="v")
                    nc.scalar.activation(v[:], z_s[:], AF.Copy,
                                         bias=1.0, scale=-1.0)  # 1 - z
                    n_t = work.tile([BL, H], f32, tag="n")
                    nc.scalar.activation(n_t[:], tn2[:], AF.Tanh)
                    if t == 0:
                        nc.vector.tensor_mul(h_new[:], v[:], n_t[:])
                    else:
                        u = work.tile([BL, H], f32, tag="u")
                        nc.vector.tensor_mul(u[:], z_s[:], h_prev[:])  # z*h
                        w1 = work.tile([BL, H], f32, tag="w1")
                        nc.vector.tensor_mul(w1[:], v[:], n_t[:])  # (1-z)*n
                        nc.vector.tensor_add(h_new[:], w1[:], u[:])
                else:
                    rz = work.tile([BL, H2], f32, tag="rz")
                    nc.scalar.activation(rz[:], G[:], AF.Sigmoid)
                    tn = work.tile([BL, H], f32, tag="tn")
                    nc.vector.tensor_mul(tn[:], rz[:, 0:H], Ghn[:])
                    tn2 = work.tile([BL, H], f32, tag="tn2")
                    nc.vector.tensor_add(tn2[:], tn[:], Gin[:])
                    n_t = work.tile([BL, H], f32, tag="n")
                    nc.scalar.activation(n_t[:], tn2[:], AF.Tanh)
                    if t == 0:
                        v = work.tile([BL, H], f32, tag="d")
                        nc.scalar.activation(v[:], rz[:, H:H2], AF.Copy,
                                             bias=1.0, scale=-1.0)  # 1 - z
                        nc.vector.tensor_mul(h_new[:], v[:], n_t[:])
                    else:
                        d = work.tile([BL, H], f32, tag="d")
                        nc.vector.tensor_sub(d[:], h_prev[:], n_t[:])
                        m = work.tile([BL, H], f32, tag="m")
                        nc.vector.tensor_mul(m[:], rz[:, H:H2], d[:])
                        nc.vector.tensor_add(h_new[:], n_t[:], m[:])
                h_prev = h_new

            def emit_fc_head():
                nonlocal h_prev, hT
                hT = transpose_h(h_prev)
                fc1 = gpsum.tile([BL, F1], f32, tag="G")
                mm(fc1[:], ones[:], bfc1[:], start=True, stop=False)
                for k in range(KC):
                    mm(fc1[:], hT[:, k * BL : (k + 1) * BL],
                       wfc1T[:, k * F1 : (k + 1) * F1],
                       start=False, stop=(k == KC - 1))
                o1 = work.tile([BL, F1], f32, tag="o1")
                nc.scalar.activation(o1[:], fc1[:], AF.Relu)

                T2 = tpsum.tile([128, 2 * BL], f32, tag="T")
                nc.tensor.transpose(T2[:, 0:BL], o1[:, 0:128], ident[:])
                nc.tensor.transpose(T2[:, BL : 2 * BL], o1[:, 128:256], ident[:])
                o1T = work.tile([128, 2 * BL], f32r, tag="o1T")
                nc.vector.tensor_copy(o1T[:], T2[:])

                fc2 = gpsum.tile([BL, C], f32, tag="Gin")
                mm(fc2[:], ones[:], bfc2[:], start=True, stop=False)
                mm(fc2[:], o1T[:, 0:BL], wfc2T[:, 0:C], start=False, stop=False)
                mm(fc2[:], o1T[:, BL : 2 * BL], wfc2T[:, C : 2 * C],
                   start=False, stop=True)
                lo = work.tile([BL, C], f32, tag="lo")
                nc.vector.tensor_copy(lo[:], fc2[:])
                nc.sync.dma_start(out=out_d.ap(), in_=lo[:])

            # bench mode: repeat the whole computation in a HW loop so the
            # per-iteration time can be extracted from noisy wall-clock.
            if reps > 1:
                with tc.For_i(0, reps, 1):
                    emit_body()
            else:
                emit_body()

    nc.compile()
    return nc


def _prep_shared(w_ih, w_hh, b_ih, b_hh, w_fc1, b_fc1, w_fc2, b_fc2):
    f = np.float32

    def kmajor(wT, kc, n):  # [kc*128, n] -> [128, kc*n]
        return np.ascontiguousarray(
            wT.reshape(kc, 128, n).transpose(1, 0, 2).reshape(128, kc * n)
        ).astype(f, copy=False)

    whhT = kmajor(np.ascontiguousarray(w_hh.T), KC, G3)
    wihT = np.ascontiguousarray(w_ih.T).astype(f, copy=False)
    b_sum = (b_ih + b_hh).astype(f)
    shared = {
        "whhT": whhT,
        "wihT": wihT,
        "brz": np.ascontiguousarray(b_sum[None, :H2]),
        "bin": np.ascontiguousarray(b_ih.astype(f)[None, H2:G3]),
        "bhn": np.ascontiguousarray(b_hh.astype(f)[None, H2:G3]),
        "ones": np.ones((1, BL), f),
        "wfc1T": kmajor(np.ascontiguousarray(w_fc1.T), KC, F1),
        "bfc1": np.ascontiguousarray(b_fc1.astype(f)[None, :]),
        "wfc2T": kmajor(np.ascontiguousarray(w_fc2.T), 2, C),
        "bfc2": np.ascontiguousarray(b_fc2.astype(f)[None, :]),
    }
    return shared


def _prep_in_maps(inputs):
    x = np.asarray(inputs["x"], dtype=np.float32)
    shared = _prep_shared(
        *(np.asarray(inputs[k], dtype=np.float32)
          for k in ("w_ih", "w_hh", "b_ih", "b_hh", "w_fc1", "b_fc1",
                    "w_fc2", "b_fc2"))
    )
    in_maps = []
    for c in range(NCORES):
        xs = x[c * BL : (c + 1) * BL]  # [64, T, I]
        xT = np.ascontiguousarray(xs.transpose(2, 1, 0).reshape(128, T * BL))
        in_maps.append({**shared, "xT": xT})
    return in_maps


def _execute(in_maps, reps=1):
    from concourse.bass_utils import run_bass_kernel_spmd

    key = ("nc", reps)
    if key not in _CACHE:
        _CACHE[key] = _build_program(reps=reps)
    nc = _CACHE[key]
    res = run_bass_kernel_spmd(nc, in_maps, core_ids=list(range(NCORES)))
    out = np.concatenate([res.results[c]["logits"] for c in range(NCORES)], axis=0)
    return out.astype(np.float32), res


def _run(inputs, trace=False, trace_kwargs=None):
    return _execute(_prep_in_maps(inputs))


def kernel(**inputs):
    out, _ = _execute(_prep_in_maps(inputs))
    return out
